# revision 45
# baseline (speedup 1.0000x reference)
"""Trainium2 Bass kernel for nn_Cross_At_50208167690358 (cosine-sim cross attention).

Math (per reference): q = x@Wq+bq; k,v = y@Wkv+bkv (split); q,k l2-normalized
over head dim (8); attn = softmax((q_hat . k_hat) * temp); out = attn @ v.
B=2, HW=4096, C=64, H=8, hd=8.

Key trick: scores s = q_hat.k_hat are cosine similarities, |s| <= 1.
Replace exp(t*s) by a degree-3 polynomial (Chebyshev interpolant of exp(t*s)
on [-1,1]) which expands exactly over the 165 monomials of degree <= 3 in the
8 head dims, collapsing softmax attention to *linear attention*:

    sum_j p(s_ij) * v_j = phi_q(q_i) . ( A @ Phi_k^T @ V_aug )

TRUNCATED feature maps: both sides keep only 128 of the 165 monomials
(deg<=2 plus deg-3 blocks d=0,1,2 sans (2,7,7); the constant is dropped --
softmax is shift-invariant).  The truncation error is repaired by a full
bilinear map A [128,128], the L2(uniform-sphere)-optimal approximation of
the kernel over the kept features, computed in closed form from monomial
moments: A = Gq^-1 Gqa diag(w(t)) Gqa^T Gq^-1 (per head temperature t).
128 features = exactly one PE-transpose chunk per i-tile and one H matmul
per column block; A folds into the tiny Mw = A @ Mt computation (1 matmul).
Accuracy vs reference on HW (incl bf16 rounding): ~7.0e-3 rel.

Sharding: 16 (b,h) units -> 2 per core (cores share batch b = core // 4).

Per-core pipeline (units fused, bf16 data, fp32 PSUM accumulation):
  A: project k/v/q in natural layout (xT/yT stationary, ones-row bias trick;
     v gets an extra all-ones column for the softmax denominator).
  B: l2-normalize: ACT square/sqrt + DVE reduce/reciprocal/mul (all ACT
     functions stay in the sqrt_and_others table -> one table load).
  C: build monomial features: broadcast-multiplies, all on DVE (the Pool
     engine measured ~4.6x slower per element and slowed the whole kernel).
  D: k-side: Mt[9,128] (per unit) += V_aug_tile^T @ Phi_k_tile  (32 matmuls).
  E: Mt -> PE-transpose -> MtT (f32); Mw[128,9] = A @ Mt (1 matmul/unit).
  F/H per unit: PE-transpose Phi_q group g (8 i-tiles) to PSUM, copy to
     SBUF (alternating DVE/ACT), while the H matmuls consume group g-1:
     out^T[9,1024] = Mw^T @ PhiT-chunk (2 512-col matmuls, 1 copy).
  I: single out DMA per unit; host does num/denom divide + relayout.
"""

import sys

if "/opt/trn_rl_repo" not in sys.path:
    sys.path.insert(0, "/opt/trn_rl_repo")

from contextlib import ExitStack
from math import factorial

import numpy as np
import ml_dtypes

import concourse.bass as bass  # noqa: F401
from concourse import bacc, mybir
import concourse.tile as tile
from concourse.bass_utils import run_bass_kernel_spmd
from concourse.masks import make_identity

P = 128
HW = 4096
C = 64
H = 8
D = 8          # head dim
B = 2
NCORES = 8
NU = 2         # (b, h) units per core
NIT = HW // P  # 32 i-tiles
NCOL = NU * NIT  # 64 fused (unit, i-tile) columns

DEG = 3
F = 165        # k-side features: 1 + 8 + 36 + 120 monomials up to degree 3
FQ = 128       # q-side features (see module docstring)
F2 = F - 128   # second k-chunk width for the A matmul (37)
IC = 1024      # H-stage column block (= one 8-i-tile transpose group)
NCH = HW // IC

F32 = mybir.dt.float32
BF16 = mybir.dt.bfloat16
AF = mybir.ActivationFunctionType

# k-side feature-block offsets (degree-2 / degree-3 prefix tables)
W2 = [8 - d for d in range(8)]                      # widths of deg-2 blocks
OFF2 = [9 + sum(W2[:d]) for d in range(8)]          # deg-2 block starts
W3 = [sum(W2[d:]) for d in range(8)]                # widths of deg-3 blocks
OFF3 = [45 + sum(W3[:d]) for d in range(8)]         # deg-3 block starts
assert OFF3[-1] + W3[-1] == F

# q-side kept monomials: deg1 (8) + deg2 (36) + deg3 blocks d=0,1,2 with the
# final (2,7,7) monomial trimmed (36+28+20).  Offsets within the 128 rows:
OFF2Q = [o - 1 for o in OFF2]                       # deg2 starts (base 8)
W3Q = [36, 28, 20]
OFF3Q = [44, 80, 108]
assert OFF3Q[-1] + W3Q[-1] == FQ
# indices of kept q features inside the 165-dim monomial order
KEPT_Q = list(range(1, 130))
KEPT_Q.remove(129)   # (2,7,7)
assert len(KEPT_Q) == FQ

_CACHE = {}


def _feat_weights(t):
    """Poly-kernel weights w_f so that sum_f w_f phi_f(q) phi_f(k) ~ exp(t*q.k)
    for unit q, k. Chebyshev interpolant of exp(t*s) on [-1,1], degree 3."""
    cheb = np.polynomial.chebyshev.chebinterpolate(
        lambda s: np.exp(t * s), DEG)
    c = np.polynomial.chebyshev.cheb2poly(cheb)

    def multinom(idx):
        counts = {}
        for d in idx:
            counts[d] = counts.get(d, 0) + 1
        r = factorial(len(idx))
        for v in counts.values():
            r //= factorial(v)
        return r

    w = np.empty(F, np.float64)
    w[0] = c[0]
    for d in range(8):
        w[1 + d] = c[1]
    i = 9
    for d1 in range(8):
        for d2 in range(d1, 8):
            w[i] = c[2] * multinom((d1, d2))
            i += 1
    for d1 in range(8):
        for d2 in range(d1, 8):
            for d3 in range(d2, 8):
                w[i] = c[3] * multinom((d1, d2, d3))
                i += 1
    assert i == F
    return w


def _monomial_exponents():
    exps = [(0,) * 8]
    for d in range(8):
        e = [0] * 8
        e[d] = 1
        exps.append(tuple(e))
    for d1 in range(8):
        for d2 in range(d1, 8):
            e = [0] * 8
            e[d1] += 1
            e[d2] += 1
            exps.append(tuple(e))
    for d1 in range(8):
        for d2 in range(d1, 8):
            for d3 in range(d2, 8):
                e = [0] * 8
                e[d1] += 1
                e[d2] += 1
                e[d3] += 1
                exps.append(tuple(e))
    return np.array(exps)


def _proj_matrix():
    """P = Gq^-1 Gqa from uniform-S^7 monomial moments; A(t) = P diag(w(t)).
    Minimizes E_{q,k uniform}[(phiq^T A phik - p_t(q.k))^2]."""
    if "projP" in _CACHE:
        return _CACHE["projP"]
    exps = _monomial_exponents()

    def dfact(n):
        r = 1
        while n > 1:
            r *= n
            n -= 2
        return r

    def moment(a):
        if any(x % 2 for x in a):
            return 0.0
        num = 1.0
        for x in a:
            num *= dfact(x - 1)
        den = 1.0
        for j in range(sum(a) // 2):
            den *= (8 + 2 * j)
        return num / den

    G = np.zeros((F, F))
    for i in range(F):
        for j in range(i, F):
            G[i, j] = G[j, i] = moment(exps[i] + exps[j])
    Gq = G[np.ix_(KEPT_Q, KEPT_Q)]
    Gqa = G[KEPT_Q, :]
    _CACHE["projP"] = np.linalg.solve(Gq, Gqa)
    return _CACHE["projP"]


def _emit_features(nc, phi, ncol=NCOL, pool_deg2=4):
    """Monomials for the 128 kept features (both sides): no const; linear at
    rows 0:8; deg2 at 8:44; deg3 blocks d=0,1,2 at 44:128 ((2,7,7) trimmed).
    deg-2 blocks d < pool_deg2 go to the Pool engine (idle but slower); the
    rest + all deg-3 go to DVE.  d descending so deg-3 block d (reading the
    deg-2 suffix from block d) can start early."""
    def hat(d):
        return phi[:, d:d + 1, 0:ncol]

    for d in reversed(range(8)):
        w = 8 - d
        eng = nc.gpsimd if d < pool_deg2 else nc.vector
        eng.tensor_mul(
            phi[:, OFF2Q[d]:OFF2Q[d] + w, 0:ncol],
            hat(d).to_broadcast((P, w, ncol)),
            phi[:, d:8, 0:ncol])
    for d in reversed(range(3)):
        w = W3Q[d]
        nc.vector.tensor_mul(
            phi[:, OFF3Q[d]:OFF3Q[d] + w, 0:ncol],
            hat(d).to_broadcast((P, w, ncol)),
            phi[:, OFF2Q[d]:OFF2Q[d] + w, 0:ncol])


def build_program(reps=1, taps=()):
    nc = bacc.Bacc("TRN2", target_bir_lowering=False, debug=False,
                   num_devices=NCORES)
    xT_d = nc.dram_tensor("xT", [C + 1, HW], BF16, kind="ExternalInput").ap()
    yT_d = nc.dram_tensor("yT", [C + 1, HW], BF16, kind="ExternalInput").ap()
    wq_d = nc.dram_tensor("wq", [C + 1, NU, D], BF16, kind="ExternalInput").ap()
    wkv_d = nc.dram_tensor("wkv", [C + 1, NU, 2 * D + 1], BF16,
                           kind="ExternalInput").ap()
    at1_d = nc.dram_tensor("at1", [P, NU, FQ], F32, kind="ExternalInput").ap()
    out_d = nc.dram_tensor("out", [NU, P, NIT, D + 1], BF16,
                           kind="ExternalOutput").ap()

    with tile.TileContext(nc) as tc, ExitStack() as ctx:
        pools = {
            "const": ctx.enter_context(tc.tile_pool(name="const", bufs=1)),
            "main": ctx.enter_context(tc.tile_pool(name="main", bufs=1)),
            "work": ctx.enter_context(tc.tile_pool(name="work", bufs=2)),
        }

        def emit_all():
            const, main, work = pools["const"], pools["main"], pools["work"]
            xT = const.tile([C + 1, HW], BF16, name="xT")
            yT = const.tile([C + 1, HW], BF16, name="yT")
            wq = const.tile([C + 1, NU, D], BF16, name="wq")
            wkv = const.tile([C + 1, NU, 2 * D + 1], BF16, name="wkv")
            at1 = const.tile([P, NU, FQ], F32, name="at1")
            identB = const.tile([P, P], BF16, name="identB")
            # weights first (tiny), then bulk inputs split across both HWDGE
            # queues (SP + ACT) in halves so projections can start early.
            nc.sync.dma_start(wkv[:], wkv_d)
            nc.sync.dma_start(wq[:], wq_d)
            nc.scalar.dma_start(at1[:], at1_d)
            HQ = HW // 4
            for qq in range(4):
                eng = nc.sync if qq % 2 == 0 else nc.scalar
                eng.dma_start(yT[:, qq * HQ:(qq + 1) * HQ],
                              yT_d[:, qq * HQ:(qq + 1) * HQ])
            for qq in range(4):
                eng = nc.scalar if qq % 2 == 0 else nc.sync
                eng.dma_start(xT[:, qq * HQ:(qq + 1) * HQ],
                              xT_d[:, qq * HQ:(qq + 1) * HQ])
            make_identity(nc, identB[:])

            # Pin the ACT function table: a no-dep Sqrt schedules first, so
            # the greedy table chooser picks sqrt_and_others (which also
            # holds Copy + Square) once instead of thrashing 1.28us reloads.
            actpin = work.tile([1, 1], F32, tag="actpin")
            nc.scalar.activation(actpin[:], identB[0:1, 0:1], AF.Sqrt)

            phiK = main.tile([P, FQ, NCOL], BF16, name="phiK")
            phiQ = main.tile([P, FQ, NCOL], BF16, name="phiQ")
            vN = main.tile([P, NU, NIT, D + 1], BF16, name="vN")
            phiT = main.tile([FQ, NU, HW], BF16, name="phiT")
            mtT_sb = main.tile([P, NU, D + 1], F32, name="mtT_sb")
            Mw = main.tile([FQ, NU, D + 1], BF16, name="Mw")

            def normalize(psv, phi_slots):
                # psv: [P, NIT, NU, 8] projection view (PSUM fp32).
                # Square/Sqrt on ACT + reduce/recip/mul on DVE: every ACT
                # function stays in the sqrt_and_others table.
                sq = work.tile([P, NIT, NU, D], F32, tag="sq")
                nc.scalar.activation(sq[:], psv, AF.Square)
                ssum = work.tile([P, NIT, NU], F32, tag="ssum")
                nc.vector.tensor_reduce(ssum[:], sq[:], mybir.AxisListType.X,
                                        mybir.AluOpType.add)
                rec = work.tile([P, NIT, NU], F32, tag="rec")
                nc.vector.reciprocal(rec[:], ssum[:])
                inv = work.tile([P, NIT, NU], F32, tag="inv")
                nc.scalar.activation(inv[:], rec[:], AF.Sqrt)
                nc.vector.tensor_mul(
                    phi_slots, psv,
                    inv[:, :, :, None].to_broadcast((P, NIT, NU, D)))

            # ---- A: projections, k first (feeds the longest chain:
            # featK -> Mt -> Mw).  k+v fused: one matmul per i-tile with
            # the yT tile stationary (34 cols: per unit 8 k + 9 v-aug).
            with tc.tile_pool(name="pkv", bufs=1, space="PSUM") as pkvp, \
                    tc.tile_pool(name="pq", bufs=1, space="PSUM") as pqp:
                ps_kv = pkvp.tile([P, NIT, 64], F32, tag="pkv", name="pskv")
                for it in range(NIT):
                    nc.tensor.matmul(
                        ps_kv[:, it, 0:NU * 17],
                        yT[:, it * P:(it + 1) * P],
                        wkv[:].rearrange("p u c -> p (u c)"),
                        start=True, stop=True)
                kvv = ps_kv[:, :, 0:NU * 17].rearrange(
                    "p it (u c) -> p it u c", u=NU)
                normalize(
                    kvv[:, :, :, 0:D],
                    phiK[:, 0:8, :].rearrange("p d (u it) -> p it u d", u=NU))

                ps_q = pqp.tile([P, NIT, NU * D], F32, tag="pq", name="psq")
                for it in range(NIT):
                    nc.tensor.matmul(
                        ps_q[:, it, :], xT[:, it * P:(it + 1) * P],
                        wq[:].rearrange("p u c -> p (u c)"),
                        start=True, stop=True)
                normalize(
                    ps_q[:].rearrange("p it (u d) -> p it u d", u=NU),
                    phiQ[:, 0:8, :].rearrange("p d (u it) -> p it u d", u=NU))

                nc.scalar.activation(
                    vN[:], kvv[:, :, :, D:].rearrange("p it u c -> p u it c"),
                    AF.Copy)

            # ---- C: monomial features (both sides: 128 kept), k first ----
            import os as _os
            _ab = _os.environ.get("ABLATE", "")
            _nc_feat = 2 if "feat" in _ab else NCOL
            _pool_deg2 = 4 if "pool" in _ab else 0
            _emit_features(nc, phiQ, ncol=_nc_feat, pool_deg2=_pool_deg2)
            _emit_features(nc, phiK, ncol=_nc_feat, pool_deg2=_pool_deg2)

            # ---- pool stack: [trp outp][smp][mtp]; peak 3+2+1+1 = 7 banks.
            fh = ExitStack()
            trp = fh.enter_context(
                tc.tile_pool(name="trp", bufs=3, space="PSUM"))
            outp = fh.enter_context(
                tc.tile_pool(name="outp", bufs=3, space="PSUM"))
            smp_cm = tc.tile_pool(name="smp", bufs=1, space="PSUM")
            smp = smp_cm.__enter__()
            mtp_cm = tc.tile_pool(name="mtp", bufs=1, space="PSUM")
            mtp = mtp_cm.__enter__()
            _skip_mt = "mt" in _ab

            # ---- F: transpose phi_q groups (featQ was built first, so this
            # PE work starts while featK still builds on DVE) ----
            for u in range(NU if "fg" not in _ab else 0):
                for g in range(4):   # groups of 8 i-tiles
                    tr = trp.tile([P, 8, P], BF16, tag="tr1", name="tr1")
                    for s in range(8):
                        it = 8 * g + s
                        nc.tensor.transpose(
                            tr[:, s, :], phiQ[:, :, u * NIT + it], identB)
                    if g % 2 == 0:
                        nc.vector.tensor_copy(
                            phiT[:, u, g * 8 * P:(g + 1) * 8 * P], tr[:])
                    else:
                        nc.scalar.activation(
                            phiT[:, u, g * 8 * P:(g + 1) * 8 * P],
                            tr[:], AF.Copy)

            # ---- D: k-side MtT[128f, 9] per unit, accumulated directly in
            # transposed layout: stationary = phiK column tile, moving = vN
            # (9 cols/matmul instead of 128) -- no E-stage transpose needed.
            mtT_ps = mtp.tile([FQ, NU, D + 1], F32, tag="mtT")
            for u in range(NU if not _skip_mt else 0):
                for it in range(NIT):
                    nc.tensor.matmul(
                        mtT_ps[:, u, :], phiK[:, :, u * NIT + it],
                        vN[:, u, it, :],
                        start=(it == 0), stop=(it == NIT - 1))
            if not _skip_mt:
                nc.vector.tensor_copy(mtT_sb[:], mtT_ps[:])
            mtp_cm.__exit__(None, None, None)
            mw_ps = smp.tile([FQ, NU, D + 1], F32, tag="mw")
            if not _skip_mt:
                for u in range(NU):
                    nc.tensor.matmul(mw_ps[:, u, :], at1[:, u, :],
                                     mtT_sb[:, u, :], start=True, stop=True)
                nc.vector.tensor_copy(Mw[:], mw_ps[:])
            smp_cm.__exit__(None, None, None)

            # ---- H in NATURAL output layout (stationary = phiT chunk,
            # moving = Mw's 9 cols): out lands on 128 partitions so the
            # PSUM->SBUF copies and the final DMA are tiny.
            outN_sb = main.tile([P, NU, NIT, D + 1], BF16, name="outN_sb")

            def emit_h(u, g):
                onat = outp.tile([P, 8, D + 1], F32, tag="onat", name="onat")
                for s in range(8):
                    it = 8 * g + s
                    nc.tensor.matmul(
                        onat[:, s, :], phiT[:, u, it * P:(it + 1) * P],
                        Mw[:, u, :], start=True, stop=True)
                dst = outN_sb[:, u, 8 * g:8 * (g + 1), :]
                # NOTE: Pool/GPSIMD cannot access PSUM on real HW
                if g % 2 == 0:
                    nc.vector.tensor_copy(dst, onat[:])
                else:
                    nc.scalar.activation(dst, onat[:], AF.Copy)

            if "fg" not in _ab and "hi" not in _ab:
                for u in range(NU):
                    for g in range(4):
                        emit_h(u, g)
                    (nc.sync if u == 0 else nc.scalar).dma_start(
                        out_d[u], outN_sb[:, u])
            fh.close()

            tap_tiles = {"phiK": phiK, "phiQ": phiQ, "vN": vN,
                         "mtT_sb": mtT_sb, "Mw": Mw, "phiT": phiT}
            for tname in taps:
                tl = tap_tiles[tname]
                td = nc.dram_tensor(f"tap_{tname}", list(tl[:].shape),
                                    tl[:].dtype, kind="ExternalOutput").ap()
                nc.sync.dma_start(td, tl[:])

        if reps == 1:
            emit_all()
        else:
            with tc.For_i(0, reps, 1):
                emit_all()

    nc.compile()
    return nc


def _prep_inputs(x, y, Wq, bq, Wkv, bkv, temperature):
    """Host-side sharding/relayout + per-head moment-fit A matrices."""
    x = np.asarray(x, np.float32)
    y = np.asarray(y, np.float32)
    Wq = np.asarray(Wq, np.float32)
    bq = np.asarray(bq, np.float32)
    Wkv = np.asarray(Wkv, np.float32)
    bkv = np.asarray(bkv, np.float32)
    temps = np.asarray(temperature, np.float32).reshape(H)
    projP = _proj_matrix()
    ones = np.ones((1, HW), dtype=np.float32)
    bf = ml_dtypes.bfloat16
    in_maps = []
    for c in range(NCORES):
        b = c // 4
        heads = [2 * (c % 4), 2 * (c % 4) + 1]
        xT = np.concatenate([np.ascontiguousarray(x[b].T), ones], 0)
        yT = np.concatenate([np.ascontiguousarray(y[b].T), ones], 0)
        wq = np.empty((C + 1, NU, D), np.float32)
        wkv = np.zeros((C + 1, NU, 2 * D + 1), np.float32)
        at1 = np.empty((P, NU, FQ), np.float32)
        for u, h in enumerate(heads):
            wq[:C, u, :] = Wq[:, D * h:D * (h + 1)]
            wq[C, u, :] = bq[D * h:D * (h + 1)]
            wkv[:C, u, 0:D] = Wkv[:, D * h:D * (h + 1)]
            wkv[C, u, 0:D] = bkv[D * h:D * (h + 1)]
            wkv[:C, u, D:2 * D] = Wkv[:, C + D * h:C + D * (h + 1)]
            wkv[C, u, D:2 * D] = bkv[C + D * h:C + D * (h + 1)]
            wkv[C, u, 2 * D] = 1.0     # ones column for the denominator
            # two-sided 128-feature bilinear map (see _proj_matrix)
            A2 = (projP * _feat_weights(float(temps[h]))[None, :]) @ projP.T
            at1[:, u, :] = A2.T.astype(np.float32)
        in_maps.append({
            "xT": xT.astype(bf), "yT": yT.astype(bf),
            "wq": wq.astype(bf), "wkv": wkv.astype(bf),
            "at1": at1,
        })
    return in_maps


def _unshard_core0(core_out):
    """core 0 raw out [NU, P, NIT, D+1] -> [HW, 16] float."""
    res = np.empty((HW, NU * D), np.float32)
    for u in range(NU):
        co = np.asarray(core_out[u]).astype(np.float32)
        co = co.transpose(1, 0, 2).reshape(HW, D + 1)
        res[:, D * u:D * (u + 1)] = co[:, :D] / co[:, D:D + 1]
    return res


def run(x, y, Wq, bq, Wkv, bkv, temperature, trace=False):
    if "nc" not in _CACHE:
        _CACHE["nc"] = build_program()
    nc = _CACHE["nc"]
    in_maps = _prep_inputs(x, y, Wq, bq, Wkv, bkv, temperature)
    res = run_bass_kernel_spmd(nc, in_maps, core_ids=list(range(NCORES)),
                               trace=trace)
    out = np.empty((B, HW, C), dtype=np.float32)
    for c in range(NCORES):
        b = c // 4
        heads = [2 * (c % 4), 2 * (c % 4) + 1]
        core_out = res.results[c]["out"]  # [NU, P, NIT, D+1]
        for u, h in enumerate(heads):
            co = np.asarray(core_out[u]).astype(np.float32)
            co = co.transpose(1, 0, 2).reshape(HW, D + 1)
            out[b, :, D * h:D * (h + 1)] = co[:, :D] / co[:, D:D + 1]
    return out, res


def kernel(x, y, Wq, bq, Wkv, bkv, temperature):
    out, _ = run(np.asarray(x), np.asarray(y), np.asarray(Wq), np.asarray(bq),
                 np.asarray(Wkv), np.asarray(bkv), np.asarray(temperature))
    return out


# revision 46
# speedup vs baseline: 1.0626x; 1.0626x over previous
"""Trainium2 Bass kernel for nn_Cross_At_50208167690358 (cosine-sim cross attention).

Math (per reference): q = x@Wq+bq; k,v = y@Wkv+bkv (split); q,k l2-normalized
over head dim (8); attn = softmax((q_hat . k_hat) * temp); out = attn @ v.
B=2, HW=4096, C=64, H=8, hd=8.

Key trick: scores s = q_hat.k_hat are cosine similarities, |s| <= 1.
Replace exp(t*s) by a degree-3 polynomial (Chebyshev interpolant of exp(t*s)
on [-1,1]) which expands exactly over the 165 monomials of degree <= 3 in the
8 head dims, collapsing softmax attention to *linear attention*:

    sum_j p(s_ij) * v_j = phi_q(q_i) . ( A @ Phi_k^T @ V_aug )

TRUNCATED feature maps: both sides keep only 128 of the 165 monomials
(deg<=2 plus deg-3 blocks d=0,1,2 sans (2,7,7); the constant is dropped --
softmax is shift-invariant).  The truncation error is repaired by a full
bilinear map A [128,128], the L2(uniform-sphere)-optimal approximation of
the kernel over the kept features, computed in closed form from monomial
moments: A = Gq^-1 Gqa diag(w(t)) Gqa^T Gq^-1 (per head temperature t).
128 features = exactly one PE-transpose chunk per i-tile and one H matmul
per column block; A folds into the tiny Mw = A @ Mt computation (1 matmul).
Accuracy vs reference on HW (incl bf16 rounding): ~7.0e-3 rel.

Sharding: 16 (b,h) units -> 2 per core (cores share batch b = core // 4).

Per-core pipeline (units fused, bf16 data, fp32 PSUM accumulation):
  A: project k/v/q in natural layout (xT/yT stationary, ones-row bias trick;
     v gets an extra all-ones column for the softmax denominator).
  B: l2-normalize: ACT square/sqrt + DVE reduce/reciprocal/mul (all ACT
     functions stay in the sqrt_and_others table -> one table load).
  C: build monomial features: broadcast-multiplies, all on DVE (the Pool
     engine measured ~4.6x slower per element and slowed the whole kernel).
  D: k-side: Mt[9,128] (per unit) += V_aug_tile^T @ Phi_k_tile  (32 matmuls).
  E: Mt -> PE-transpose -> MtT (f32); Mw[128,9] = A @ Mt (1 matmul/unit).
  F/H per unit: PE-transpose Phi_q group g (8 i-tiles) to PSUM, copy to
     SBUF (alternating DVE/ACT), while the H matmuls consume group g-1:
     out^T[9,1024] = Mw^T @ PhiT-chunk (2 512-col matmuls, 1 copy).
  I: single out DMA per unit; host does num/denom divide + relayout.
"""

import sys

if "/opt/trn_rl_repo" not in sys.path:
    sys.path.insert(0, "/opt/trn_rl_repo")

from contextlib import ExitStack
from math import factorial

import numpy as np
import ml_dtypes

import concourse.bass as bass  # noqa: F401
from concourse import bacc, mybir
import concourse.tile as tile
from concourse.bass_utils import run_bass_kernel_spmd
from concourse.masks import make_identity

P = 128
HW = 4096
C = 64
H = 8
D = 8          # head dim
B = 2
NCORES = 8
NU = 2         # (b, h) units per core
NIT = HW // P  # 32 i-tiles
NCOL = NU * NIT  # 64 fused (unit, i-tile) columns

DEG = 3
F = 165        # k-side features: 1 + 8 + 36 + 120 monomials up to degree 3
FQ = 128       # q-side features (see module docstring)
F2 = F - 128   # second k-chunk width for the A matmul (37)
IC = 1024      # H-stage column block (= one 8-i-tile transpose group)
NCH = HW // IC

F32 = mybir.dt.float32
BF16 = mybir.dt.bfloat16
AF = mybir.ActivationFunctionType

# k-side feature-block offsets (degree-2 / degree-3 prefix tables)
W2 = [8 - d for d in range(8)]                      # widths of deg-2 blocks
OFF2 = [9 + sum(W2[:d]) for d in range(8)]          # deg-2 block starts
W3 = [sum(W2[d:]) for d in range(8)]                # widths of deg-3 blocks
OFF3 = [45 + sum(W3[:d]) for d in range(8)]         # deg-3 block starts
assert OFF3[-1] + W3[-1] == F

# q-side kept monomials: deg1 (8) + deg2 (36) + deg3 blocks d=0,1,2 with the
# final (2,7,7) monomial trimmed (36+28+20).  Offsets within the 128 rows:
OFF2Q = [o - 1 for o in OFF2]                       # deg2 starts (base 8)
W3Q = [36, 28, 20]
OFF3Q = [44, 80, 108]
assert OFF3Q[-1] + W3Q[-1] == FQ
# indices of kept q features inside the 165-dim monomial order
KEPT_Q = list(range(1, 130))
KEPT_Q.remove(129)   # (2,7,7)
assert len(KEPT_Q) == FQ

_CACHE = {}


def _feat_weights(t):
    """Poly-kernel weights w_f so that sum_f w_f phi_f(q) phi_f(k) ~ exp(t*q.k)
    for unit q, k. Chebyshev interpolant of exp(t*s) on [-1,1], degree 3."""
    cheb = np.polynomial.chebyshev.chebinterpolate(
        lambda s: np.exp(t * s), DEG)
    c = np.polynomial.chebyshev.cheb2poly(cheb)

    def multinom(idx):
        counts = {}
        for d in idx:
            counts[d] = counts.get(d, 0) + 1
        r = factorial(len(idx))
        for v in counts.values():
            r //= factorial(v)
        return r

    w = np.empty(F, np.float64)
    w[0] = c[0]
    for d in range(8):
        w[1 + d] = c[1]
    i = 9
    for d1 in range(8):
        for d2 in range(d1, 8):
            w[i] = c[2] * multinom((d1, d2))
            i += 1
    for d1 in range(8):
        for d2 in range(d1, 8):
            for d3 in range(d2, 8):
                w[i] = c[3] * multinom((d1, d2, d3))
                i += 1
    assert i == F
    return w


def _monomial_exponents():
    exps = [(0,) * 8]
    for d in range(8):
        e = [0] * 8
        e[d] = 1
        exps.append(tuple(e))
    for d1 in range(8):
        for d2 in range(d1, 8):
            e = [0] * 8
            e[d1] += 1
            e[d2] += 1
            exps.append(tuple(e))
    for d1 in range(8):
        for d2 in range(d1, 8):
            for d3 in range(d2, 8):
                e = [0] * 8
                e[d1] += 1
                e[d2] += 1
                e[d3] += 1
                exps.append(tuple(e))
    return np.array(exps)


def _proj_matrix():
    """P = Gq^-1 Gqa from uniform-S^7 monomial moments; A(t) = P diag(w(t)).
    Minimizes E_{q,k uniform}[(phiq^T A phik - p_t(q.k))^2]."""
    if "projP" in _CACHE:
        return _CACHE["projP"]
    exps = _monomial_exponents()

    def dfact(n):
        r = 1
        while n > 1:
            r *= n
            n -= 2
        return r

    def moment(a):
        if any(x % 2 for x in a):
            return 0.0
        num = 1.0
        for x in a:
            num *= dfact(x - 1)
        den = 1.0
        for j in range(sum(a) // 2):
            den *= (8 + 2 * j)
        return num / den

    G = np.zeros((F, F))
    for i in range(F):
        for j in range(i, F):
            G[i, j] = G[j, i] = moment(exps[i] + exps[j])
    Gq = G[np.ix_(KEPT_Q, KEPT_Q)]
    Gqa = G[KEPT_Q, :]
    _CACHE["projP"] = np.linalg.solve(Gq, Gqa)
    return _CACHE["projP"]


def _emit_features(nc, phi, ncol=NCOL, pool_deg2=4):
    """Monomials for the 128 kept features (both sides): no const; linear at
    rows 0:8; deg2 at 8:44; deg3 blocks d=0,1,2 at 44:128 ((2,7,7) trimmed).
    deg-2 blocks d < pool_deg2 go to the Pool engine (idle but slower); the
    rest + all deg-3 go to DVE.  d descending so deg-3 block d (reading the
    deg-2 suffix from block d) can start early."""
    def hat(d):
        return phi[:, d:d + 1, 0:ncol]

    for d in reversed(range(8)):
        w = 8 - d
        eng = nc.gpsimd if d < pool_deg2 else nc.vector
        eng.tensor_mul(
            phi[:, OFF2Q[d]:OFF2Q[d] + w, 0:ncol],
            hat(d).to_broadcast((P, w, ncol)),
            phi[:, d:8, 0:ncol])
    for d in reversed(range(3)):
        w = W3Q[d]
        nc.vector.tensor_mul(
            phi[:, OFF3Q[d]:OFF3Q[d] + w, 0:ncol],
            hat(d).to_broadcast((P, w, ncol)),
            phi[:, OFF2Q[d]:OFF2Q[d] + w, 0:ncol])


def build_program(reps=1, taps=()):
    nc = bacc.Bacc("TRN2", target_bir_lowering=False, debug=False,
                   num_devices=NCORES)
    xT_d = nc.dram_tensor("xT", [C + 1, HW], BF16, kind="ExternalInput").ap()
    yT_d = nc.dram_tensor("yT", [C + 1, HW], BF16, kind="ExternalInput").ap()
    wq_d = nc.dram_tensor("wq", [C + 1, NU, D], BF16, kind="ExternalInput").ap()
    wkv_d = nc.dram_tensor("wkv", [C + 1, NU, 2 * D + 1], BF16,
                           kind="ExternalInput").ap()
    at1_d = nc.dram_tensor("at1", [P, NU, FQ], F32, kind="ExternalInput").ap()
    out_d = nc.dram_tensor("out", [NU, P, NIT, D + 1], BF16,
                           kind="ExternalOutput").ap()

    with tile.TileContext(nc) as tc, ExitStack() as ctx:
        pools = {
            "const": ctx.enter_context(tc.tile_pool(name="const", bufs=1)),
            "main": ctx.enter_context(tc.tile_pool(name="main", bufs=1)),
            "work": ctx.enter_context(tc.tile_pool(name="work", bufs=2)),
        }

        def emit_all():
            const, main, work = pools["const"], pools["main"], pools["work"]
            xT = const.tile([C + 1, HW], BF16, name="xT")
            yT = const.tile([C + 1, HW], BF16, name="yT")
            wq = const.tile([C + 1, NU, D], BF16, name="wq")
            wkv = const.tile([C + 1, NU, 2 * D + 1], BF16, name="wkv")
            at1 = const.tile([P, NU, FQ], F32, name="at1")
            identB = const.tile([P, P], BF16, name="identB")
            # weights first (tiny), then bulk inputs split across both HWDGE
            # queues (SP + ACT) in halves so projections can start early.
            nc.sync.dma_start(wkv[:], wkv_d)
            nc.sync.dma_start(wq[:], wq_d)
            nc.scalar.dma_start(at1[:], at1_d)
            HQ = HW // 4
            for qq in range(4):
                eng = nc.sync if qq % 2 == 0 else nc.scalar
                eng.dma_start(yT[:, qq * HQ:(qq + 1) * HQ],
                              yT_d[:, qq * HQ:(qq + 1) * HQ])
            for qq in range(4):
                eng = nc.scalar if qq % 2 == 0 else nc.sync
                eng.dma_start(xT[:, qq * HQ:(qq + 1) * HQ],
                              xT_d[:, qq * HQ:(qq + 1) * HQ])
            make_identity(nc, identB[:])

            # Pin the ACT function table: a no-dep Sqrt schedules first, so
            # the greedy table chooser picks sqrt_and_others (which also
            # holds Copy + Square) once instead of thrashing 1.28us reloads.
            actpin = work.tile([1, 1], F32, tag="actpin")
            nc.scalar.activation(actpin[:], identB[0:1, 0:1], AF.Sqrt)

            phiK = main.tile([P, FQ, NCOL], BF16, name="phiK")
            phiQ = main.tile([P, FQ, NCOL], BF16, name="phiQ")
            vN = main.tile([P, NU, NIT, D + 1], BF16, name="vN")
            phiT = main.tile([FQ, NU, HW], BF16, name="phiT")
            mtT_sb = main.tile([P, NU, D + 1], F32, name="mtT_sb")
            Mw = main.tile([FQ, NU, D + 1], BF16, name="Mw")

            def normalize(psv, phi_slots):
                # psv: [P, NIT, NU, 8] projection view (PSUM fp32).
                # Square/Sqrt on ACT + reduce/recip/mul on DVE: every ACT
                # function stays in the sqrt_and_others table.
                sq = work.tile([P, NIT, NU, D], F32, tag="sq")
                nc.scalar.activation(sq[:], psv, AF.Square)
                ssum = work.tile([P, NIT, NU], F32, tag="ssum")
                nc.vector.tensor_reduce(ssum[:], sq[:], mybir.AxisListType.X,
                                        mybir.AluOpType.add)
                rec = work.tile([P, NIT, NU], F32, tag="rec")
                nc.vector.reciprocal(rec[:], ssum[:])
                inv = work.tile([P, NIT, NU], F32, tag="inv")
                nc.scalar.activation(inv[:], rec[:], AF.Sqrt)
                nc.vector.tensor_mul(
                    phi_slots, psv,
                    inv[:, :, :, None].to_broadcast((P, NIT, NU, D)))

            # ---- A: projections, k first (feeds the longest chain:
            # featK -> Mt -> Mw).  k+v fused: one matmul per i-tile with
            # the yT tile stationary (34 cols: per unit 8 k + 9 v-aug).
            with tc.tile_pool(name="pkv", bufs=1, space="PSUM") as pkvp, \
                    tc.tile_pool(name="pq", bufs=1, space="PSUM") as pqp:
                ps_kv = pkvp.tile([P, NIT, 64], F32, tag="pkv", name="pskv")
                for it in range(NIT):
                    nc.tensor.matmul(
                        ps_kv[:, it, 0:NU * 17],
                        yT[:, it * P:(it + 1) * P],
                        wkv[:].rearrange("p u c -> p (u c)"),
                        start=True, stop=True)
                kvv = ps_kv[:, :, 0:NU * 17].rearrange(
                    "p it (u c) -> p it u c", u=NU)
                normalize(
                    kvv[:, :, :, 0:D],
                    phiK[:, 0:8, :].rearrange("p d (u it) -> p it u d", u=NU))

                ps_q = pqp.tile([P, NIT, NU * D], F32, tag="pq", name="psq")
                for it in range(NIT):
                    nc.tensor.matmul(
                        ps_q[:, it, :], xT[:, it * P:(it + 1) * P],
                        wq[:].rearrange("p u c -> p (u c)"),
                        start=True, stop=True)
                normalize(
                    ps_q[:].rearrange("p it (u d) -> p it u d", u=NU),
                    phiQ[:, 0:8, :].rearrange("p d (u it) -> p it u d", u=NU))

                nc.scalar.activation(
                    vN[:], kvv[:, :, :, D:].rearrange("p it u c -> p u it c"),
                    AF.Copy)

            # ---- C: monomial features (both sides: 128 kept), k first ----
            import os as _os
            _ab = _os.environ.get("ABLATE", "")
            _nc_feat = 2 if "feat" in _ab else NCOL
            _pool_deg2 = 4 if "pool" in _ab else 0
            _emit_features(nc, phiK, ncol=_nc_feat, pool_deg2=_pool_deg2)
            _emit_features(nc, phiQ, ncol=_nc_feat, pool_deg2=_pool_deg2)

            # ---- pool stack: [trp outp][smp][mtp]; peak 3+2+1+1 = 7 banks.
            fh = ExitStack()
            trp = fh.enter_context(
                tc.tile_pool(name="trp", bufs=3, space="PSUM"))
            outp = fh.enter_context(
                tc.tile_pool(name="outp", bufs=3, space="PSUM"))
            smp_cm = tc.tile_pool(name="smp", bufs=1, space="PSUM")
            smp = smp_cm.__enter__()
            mtp_cm = tc.tile_pool(name="mtp", bufs=1, space="PSUM")
            mtp = mtp_cm.__enter__()
            _skip_mt = "mt" in _ab

            # D: k-side MtT[128f, 9] per unit, accumulated directly in
            # transposed layout: stationary = phiK column tile, moving = vN
            # (9 cols/matmul instead of 128) -- no E-stage transpose needed.
            mtT_ps = mtp.tile([FQ, NU, D + 1], F32, tag="mtT")
            for u in range(NU if not _skip_mt else 0):
                for it in range(NIT):
                    nc.tensor.matmul(
                        mtT_ps[:, u, :], phiK[:, :, u * NIT + it],
                        vN[:, u, it, :],
                        start=(it == 0), stop=(it == NIT - 1))
            if not _skip_mt:
                nc.vector.tensor_copy(mtT_sb[:], mtT_ps[:])
            mtp_cm.__exit__(None, None, None)
            mw_ps = smp.tile([FQ, NU, D + 1], F32, tag="mw")

            def emit_mw():
                # deferred until after the first transpose group so the PE
                # doesn't stall on the DVE mtT copy with transposes queued
                if not _skip_mt:
                    for u in range(NU):
                        nc.tensor.matmul(mw_ps[:, u, :], at1[:, u, :],
                                         mtT_sb[:, u, :], start=True,
                                         stop=True)
                    nc.vector.tensor_copy(Mw[:], mw_ps[:])
                smp_cm.__exit__(None, None, None)

            # ---- F/H fused per unit: transpose group g of phi_q (8 i-tiles)
            # while H consumes group g-1 in NATURAL output layout (stationary
            # = phiT chunk, moving = Mw's 9 cols): out lands on 128
            # partitions so the PSUM->SBUF copies and the final DMA are tiny.
            outN_sb = main.tile([P, NU, NIT, D + 1], BF16, name="outN_sb")

            def emit_h(u, g):
                onat = outp.tile([P, 8, D + 1], F32, tag="onat", name="onat")
                for s in range(8):
                    it = 8 * g + s
                    nc.tensor.matmul(
                        onat[:, s, :], phiT[:, u, it * P:(it + 1) * P],
                        Mw[:, u, :], start=True, stop=True)
                dst = outN_sb[:, u, 8 * g:8 * (g + 1), :]
                # NOTE: Pool/GPSIMD cannot access PSUM on real HW
                if g % 2 == 0:
                    nc.vector.tensor_copy(dst, onat[:])
                else:
                    nc.scalar.activation(dst, onat[:], AF.Copy)

            for u in range(NU if "fg" not in _ab else 0):
                for g in range(4):   # groups of 8 i-tiles
                    tr = trp.tile([P, 8, P], BF16, tag="tr1", name="tr1")
                    for s in range(8):
                        it = 8 * g + s
                        nc.tensor.transpose(
                            tr[:, s, :], phiQ[:, :, u * NIT + it], identB)
                    if g % 2 == 0:
                        nc.vector.tensor_copy(
                            phiT[:, u, g * 8 * P:(g + 1) * 8 * P], tr[:])
                    else:
                        nc.scalar.activation(
                            phiT[:, u, g * 8 * P:(g + 1) * 8 * P],
                            tr[:], AF.Copy)
                    if u == 0 and g == 0:
                        emit_mw()
                    if g >= 1 and "hi" not in _ab:
                        emit_h(u, g - 1)
                if "hi" not in _ab:
                    emit_h(u, 3)
                    (nc.sync if u == 0 else nc.scalar).dma_start(
                        out_d[u], outN_sb[:, u])
            if "fg" in _ab:
                emit_mw()
            fh.close()

            tap_tiles = {"phiK": phiK, "phiQ": phiQ, "vN": vN,
                         "mtT_sb": mtT_sb, "Mw": Mw, "phiT": phiT}
            for tname in taps:
                tl = tap_tiles[tname]
                td = nc.dram_tensor(f"tap_{tname}", list(tl[:].shape),
                                    tl[:].dtype, kind="ExternalOutput").ap()
                nc.sync.dma_start(td, tl[:])

        if reps == 1:
            emit_all()
        else:
            with tc.For_i(0, reps, 1):
                emit_all()

    nc.compile()
    return nc


def _prep_inputs(x, y, Wq, bq, Wkv, bkv, temperature):
    """Host-side sharding/relayout + per-head moment-fit A matrices."""
    x = np.asarray(x, np.float32)
    y = np.asarray(y, np.float32)
    Wq = np.asarray(Wq, np.float32)
    bq = np.asarray(bq, np.float32)
    Wkv = np.asarray(Wkv, np.float32)
    bkv = np.asarray(bkv, np.float32)
    temps = np.asarray(temperature, np.float32).reshape(H)
    projP = _proj_matrix()
    ones = np.ones((1, HW), dtype=np.float32)
    bf = ml_dtypes.bfloat16
    in_maps = []
    for c in range(NCORES):
        b = c // 4
        heads = [2 * (c % 4), 2 * (c % 4) + 1]
        xT = np.concatenate([np.ascontiguousarray(x[b].T), ones], 0)
        yT = np.concatenate([np.ascontiguousarray(y[b].T), ones], 0)
        wq = np.empty((C + 1, NU, D), np.float32)
        wkv = np.zeros((C + 1, NU, 2 * D + 1), np.float32)
        at1 = np.empty((P, NU, FQ), np.float32)
        for u, h in enumerate(heads):
            wq[:C, u, :] = Wq[:, D * h:D * (h + 1)]
            wq[C, u, :] = bq[D * h:D * (h + 1)]
            wkv[:C, u, 0:D] = Wkv[:, D * h:D * (h + 1)]
            wkv[C, u, 0:D] = bkv[D * h:D * (h + 1)]
            wkv[:C, u, D:2 * D] = Wkv[:, C + D * h:C + D * (h + 1)]
            wkv[C, u, D:2 * D] = bkv[C + D * h:C + D * (h + 1)]
            wkv[C, u, 2 * D] = 1.0     # ones column for the denominator
            # two-sided 128-feature bilinear map (see _proj_matrix)
            A2 = (projP * _feat_weights(float(temps[h]))[None, :]) @ projP.T
            at1[:, u, :] = A2.T.astype(np.float32)
        in_maps.append({
            "xT": xT.astype(bf), "yT": yT.astype(bf),
            "wq": wq.astype(bf), "wkv": wkv.astype(bf),
            "at1": at1,
        })
    return in_maps


def _unshard_core0(core_out):
    """core 0 raw out [NU, P, NIT, D+1] -> [HW, 16] float."""
    res = np.empty((HW, NU * D), np.float32)
    for u in range(NU):
        co = np.asarray(core_out[u]).astype(np.float32)
        co = co.transpose(1, 0, 2).reshape(HW, D + 1)
        res[:, D * u:D * (u + 1)] = co[:, :D] / co[:, D:D + 1]
    return res


def run(x, y, Wq, bq, Wkv, bkv, temperature, trace=False):
    if "nc" not in _CACHE:
        _CACHE["nc"] = build_program()
    nc = _CACHE["nc"]
    in_maps = _prep_inputs(x, y, Wq, bq, Wkv, bkv, temperature)
    res = run_bass_kernel_spmd(nc, in_maps, core_ids=list(range(NCORES)),
                               trace=trace)
    out = np.empty((B, HW, C), dtype=np.float32)
    for c in range(NCORES):
        b = c // 4
        heads = [2 * (c % 4), 2 * (c % 4) + 1]
        core_out = res.results[c]["out"]  # [NU, P, NIT, D+1]
        for u, h in enumerate(heads):
            co = np.asarray(core_out[u]).astype(np.float32)
            co = co.transpose(1, 0, 2).reshape(HW, D + 1)
            out[b, :, D * h:D * (h + 1)] = co[:, :D] / co[:, D:D + 1]
    return out, res


def kernel(x, y, Wq, bq, Wkv, bkv, temperature):
    out, _ = run(np.asarray(x), np.asarray(y), np.asarray(Wq), np.asarray(bq),
                 np.asarray(Wkv), np.asarray(bkv), np.asarray(temperature))
    return out


# revision 47
# speedup vs baseline: 1.1260x; 1.0596x over previous
"""Trainium2 Bass kernel for nn_Cross_At_50208167690358 (cosine-sim cross attention).

Math (per reference): q = x@Wq+bq; k,v = y@Wkv+bkv (split); q,k l2-normalized
over head dim (8); attn = softmax((q_hat . k_hat) * temp); out = attn @ v.
B=2, HW=4096, C=64, H=8, hd=8.

Key trick: scores s = q_hat.k_hat are cosine similarities, |s| <= 1.
Replace exp(t*s) by a degree-3 polynomial (Chebyshev interpolant of exp(t*s)
on [-1,1]) which expands exactly over the 165 monomials of degree <= 3 in the
8 head dims, collapsing softmax attention to *linear attention*:

    sum_j p(s_ij) * v_j = phi_q(q_i) . ( A @ Phi_k^T @ V_aug )

TRUNCATED feature maps: both sides keep only 128 of the 165 monomials
(deg<=2 plus deg-3 blocks d=0,1,2 sans (2,7,7); the constant is dropped --
softmax is shift-invariant).  The truncation error is repaired by a full
bilinear map A [128,128], the L2(uniform-sphere)-optimal approximation of
the kernel over the kept features, computed in closed form from monomial
moments: A = Gq^-1 Gqa diag(w(t)) Gqa^T Gq^-1 (per head temperature t).
128 features = exactly one PE-transpose chunk per i-tile and one H matmul
per column block; A folds into the tiny Mw = A @ Mt computation (1 matmul).
Accuracy vs reference on HW (incl bf16 rounding): ~7.0e-3 rel.

Sharding: 16 (b,h) units -> 2 per core (cores share batch b = core // 4).

Per-core pipeline (units fused, bf16 data, fp32 PSUM accumulation):
  A: project k/v/q in natural layout (xT/yT stationary, ones-row bias trick;
     v gets an extra all-ones column for the softmax denominator).
  B: l2-normalize: ACT square/sqrt + DVE reduce/reciprocal/mul (all ACT
     functions stay in the sqrt_and_others table -> one table load).
  C: build monomial features: broadcast-multiplies, all on DVE (the Pool
     engine measured ~4.6x slower per element and slowed the whole kernel).
  D: k-side: Mt[9,128] (per unit) += V_aug_tile^T @ Phi_k_tile  (32 matmuls).
  E: Mt -> PE-transpose -> MtT (f32); Mw[128,9] = A @ Mt (1 matmul/unit).
  F/H per unit: PE-transpose Phi_q group g (8 i-tiles) to PSUM, copy to
     SBUF (alternating DVE/ACT), while the H matmuls consume group g-1:
     out^T[9,1024] = Mw^T @ PhiT-chunk (2 512-col matmuls, 1 copy).
  I: single out DMA per unit; host does num/denom divide + relayout.
"""

import sys

if "/opt/trn_rl_repo" not in sys.path:
    sys.path.insert(0, "/opt/trn_rl_repo")

from contextlib import ExitStack
from math import factorial

import numpy as np
import ml_dtypes

import concourse.bass as bass  # noqa: F401
from concourse import bacc, mybir
import concourse.tile as tile
from concourse.bass_utils import run_bass_kernel_spmd
from concourse.masks import make_identity

P = 128
HW = 4096
C = 64
H = 8
D = 8          # head dim
B = 2
NCORES = 8
NU = 2         # (b, h) units per core
NIT = HW // P  # 32 i-tiles
NCOL = NU * NIT  # 64 fused (unit, i-tile) columns

DEG = 3
F = 165        # k-side features: 1 + 8 + 36 + 120 monomials up to degree 3
FQ = 128       # q-side features (see module docstring)
F2 = F - 128   # second k-chunk width for the A matmul (37)
IC = 1024      # H-stage column block (= one 8-i-tile transpose group)
NCH = HW // IC

F32 = mybir.dt.float32
BF16 = mybir.dt.bfloat16
AF = mybir.ActivationFunctionType

# k-side feature-block offsets (degree-2 / degree-3 prefix tables)
W2 = [8 - d for d in range(8)]                      # widths of deg-2 blocks
OFF2 = [9 + sum(W2[:d]) for d in range(8)]          # deg-2 block starts
W3 = [sum(W2[d:]) for d in range(8)]                # widths of deg-3 blocks
OFF3 = [45 + sum(W3[:d]) for d in range(8)]         # deg-3 block starts
assert OFF3[-1] + W3[-1] == F

# q-side kept monomials: deg1 (8) + deg2 (36) + deg3 blocks d=0,1,2 with the
# final (2,7,7) monomial trimmed (36+28+20).  Offsets within the 128 rows:
OFF2Q = [o - 1 for o in OFF2]                       # deg2 starts (base 8)
W3Q = [36, 28, 20]
OFF3Q = [44, 80, 108]
assert OFF3Q[-1] + W3Q[-1] == FQ
# indices of kept q features inside the 165-dim monomial order
KEPT_Q = list(range(1, 130))
KEPT_Q.remove(129)   # (2,7,7)
assert len(KEPT_Q) == FQ

_CACHE = {}


def _feat_weights(t):
    """Poly-kernel weights w_f so that sum_f w_f phi_f(q) phi_f(k) ~ exp(t*q.k)
    for unit q, k. Chebyshev interpolant of exp(t*s) on [-1,1], degree 3."""
    cheb = np.polynomial.chebyshev.chebinterpolate(
        lambda s: np.exp(t * s), DEG)
    c = np.polynomial.chebyshev.cheb2poly(cheb)

    def multinom(idx):
        counts = {}
        for d in idx:
            counts[d] = counts.get(d, 0) + 1
        r = factorial(len(idx))
        for v in counts.values():
            r //= factorial(v)
        return r

    w = np.empty(F, np.float64)
    w[0] = c[0]
    for d in range(8):
        w[1 + d] = c[1]
    i = 9
    for d1 in range(8):
        for d2 in range(d1, 8):
            w[i] = c[2] * multinom((d1, d2))
            i += 1
    for d1 in range(8):
        for d2 in range(d1, 8):
            for d3 in range(d2, 8):
                w[i] = c[3] * multinom((d1, d2, d3))
                i += 1
    assert i == F
    return w


def _monomial_exponents():
    exps = [(0,) * 8]
    for d in range(8):
        e = [0] * 8
        e[d] = 1
        exps.append(tuple(e))
    for d1 in range(8):
        for d2 in range(d1, 8):
            e = [0] * 8
            e[d1] += 1
            e[d2] += 1
            exps.append(tuple(e))
    for d1 in range(8):
        for d2 in range(d1, 8):
            for d3 in range(d2, 8):
                e = [0] * 8
                e[d1] += 1
                e[d2] += 1
                e[d3] += 1
                exps.append(tuple(e))
    return np.array(exps)


def _proj_matrix():
    """P = Gq^-1 Gqa from uniform-S^7 monomial moments; A(t) = P diag(w(t)).
    Minimizes E_{q,k uniform}[(phiq^T A phik - p_t(q.k))^2]."""
    if "projP" in _CACHE:
        return _CACHE["projP"]
    exps = _monomial_exponents()

    def dfact(n):
        r = 1
        while n > 1:
            r *= n
            n -= 2
        return r

    def moment(a):
        if any(x % 2 for x in a):
            return 0.0
        num = 1.0
        for x in a:
            num *= dfact(x - 1)
        den = 1.0
        for j in range(sum(a) // 2):
            den *= (8 + 2 * j)
        return num / den

    G = np.zeros((F, F))
    for i in range(F):
        for j in range(i, F):
            G[i, j] = G[j, i] = moment(exps[i] + exps[j])
    Gq = G[np.ix_(KEPT_Q, KEPT_Q)]
    Gqa = G[KEPT_Q, :]
    _CACHE["projP"] = np.linalg.solve(Gq, Gqa)
    return _CACHE["projP"]


def _emit_features(nc, phi, ncol=NCOL, pool_deg2=4):
    """Monomials for the 128 kept features (both sides): no const; linear at
    rows 0:8; deg2 at 8:44; deg3 blocks d=0,1,2 at 44:128 ((2,7,7) trimmed).
    deg-2 blocks d < pool_deg2 go to the Pool engine (idle but slower); the
    rest + all deg-3 go to DVE.  d descending so deg-3 block d (reading the
    deg-2 suffix from block d) can start early."""
    def hat(d):
        return phi[:, d:d + 1, 0:ncol]

    for d in reversed(range(8)):
        w = 8 - d
        eng = nc.gpsimd if d < pool_deg2 else nc.vector
        eng.tensor_mul(
            phi[:, OFF2Q[d]:OFF2Q[d] + w, 0:ncol],
            hat(d).to_broadcast((P, w, ncol)),
            phi[:, d:8, 0:ncol])
    for d in reversed(range(3)):
        w = W3Q[d]
        nc.vector.tensor_mul(
            phi[:, OFF3Q[d]:OFF3Q[d] + w, 0:ncol],
            hat(d).to_broadcast((P, w, ncol)),
            phi[:, OFF2Q[d]:OFF2Q[d] + w, 0:ncol])


def build_program(reps=1, taps=()):
    nc = bacc.Bacc("TRN2", target_bir_lowering=False, debug=False,
                   num_devices=NCORES)
    xT_d = nc.dram_tensor("xT", [C + 1, HW], BF16, kind="ExternalInput").ap()
    yT_d = nc.dram_tensor("yT", [C + 1, HW], BF16, kind="ExternalInput").ap()
    wq_d = nc.dram_tensor("wq", [C + 1, NU, D], BF16, kind="ExternalInput").ap()
    wkv_d = nc.dram_tensor("wkv", [C + 1, NU, 2 * D + 1], BF16,
                           kind="ExternalInput").ap()
    at1_d = nc.dram_tensor("at1", [P, NU, FQ], F32, kind="ExternalInput").ap()
    out_d = nc.dram_tensor("out", [NU, P, NIT, D + 1], BF16,
                           kind="ExternalOutput").ap()

    with tile.TileContext(nc) as tc, ExitStack() as ctx:
        pools = {
            "const": ctx.enter_context(tc.tile_pool(name="const", bufs=1)),
            "main": ctx.enter_context(tc.tile_pool(name="main", bufs=1)),
            "work": ctx.enter_context(tc.tile_pool(name="work", bufs=2)),
        }

        def emit_all():
            const, main, work = pools["const"], pools["main"], pools["work"]
            xT = const.tile([C + 1, HW], BF16, name="xT")
            yT = const.tile([C + 1, HW], BF16, name="yT")
            wq = const.tile([C + 1, NU, D], BF16, name="wq")
            wkv = const.tile([C + 1, NU, 2 * D + 1], BF16, name="wkv")
            at1 = const.tile([P, NU, FQ], F32, name="at1")
            identB = const.tile([P, P], BF16, name="identB")
            # weights first (tiny), then bulk inputs split across both HWDGE
            # queues (SP + ACT) in halves so projections can start early.
            nc.sync.dma_start(wkv[:], wkv_d)
            nc.sync.dma_start(wq[:], wq_d)
            nc.scalar.dma_start(at1[:], at1_d)
            HQ = HW // 4
            for qq in range(4):
                eng = nc.sync if qq % 2 == 0 else nc.scalar
                eng.dma_start(yT[:, qq * HQ:(qq + 1) * HQ],
                              yT_d[:, qq * HQ:(qq + 1) * HQ])
            for qq in range(4):
                eng = nc.scalar if qq % 2 == 0 else nc.sync
                eng.dma_start(xT[:, qq * HQ:(qq + 1) * HQ],
                              xT_d[:, qq * HQ:(qq + 1) * HQ])
            make_identity(nc, identB[:])

            # Pin the ACT function table: a no-dep Sqrt schedules first, so
            # the greedy table chooser picks sqrt_and_others (which also
            # holds Copy + Square) once instead of thrashing 1.28us reloads.
            actpin = work.tile([1, 1], F32, tag="actpin")
            nc.scalar.activation(actpin[:], identB[0:1, 0:1], AF.Sqrt)

            phiK = main.tile([P, FQ, NCOL], BF16, name="phiK")
            phiQ = main.tile([P, FQ, NCOL], BF16, name="phiQ")
            vN = main.tile([P, NU, NIT, D + 1], BF16, name="vN")
            phiT = main.tile([FQ, NU, HW], BF16, name="phiT")
            mtT_sb = main.tile([P, NU, D + 1], F32, name="mtT_sb")
            Mw = main.tile([FQ, NU, D + 1], BF16, name="Mw")

            def normalize(psv, phi_slots):
                # psv: [P, NIT, NU, 8] projection view (PSUM fp32).
                # Square/Sqrt on ACT + reduce/recip/mul on DVE: every ACT
                # function stays in the sqrt_and_others table.
                sq = work.tile([P, NIT, NU, D], F32, tag="sq")
                nc.scalar.activation(sq[:], psv, AF.Square)
                ssum = work.tile([P, NIT, NU], F32, tag="ssum")
                nc.vector.tensor_reduce(ssum[:], sq[:], mybir.AxisListType.X,
                                        mybir.AluOpType.add)
                rec = work.tile([P, NIT, NU], F32, tag="rec")
                nc.vector.reciprocal(rec[:], ssum[:])
                inv = work.tile([P, NIT, NU], F32, tag="inv")
                nc.scalar.activation(inv[:], rec[:], AF.Sqrt)
                nc.vector.tensor_mul(
                    phi_slots, psv,
                    inv[:, :, :, None].to_broadcast((P, NIT, NU, D)))

            # ---- A: projections, k first (feeds the longest chain:
            # featK -> Mt -> Mw).  k+v fused: one matmul per i-tile with
            # the yT tile stationary (34 cols: per unit 8 k + 9 v-aug).
            with tc.tile_pool(name="pkv", bufs=1, space="PSUM") as pkvp, \
                    tc.tile_pool(name="pq", bufs=1, space="PSUM") as pqp:
                ps_kv = pkvp.tile([P, NIT, 64], F32, tag="pkv", name="pskv")
                for it in range(NIT):
                    nc.tensor.matmul(
                        ps_kv[:, it, 0:NU * 17],
                        yT[:, it * P:(it + 1) * P],
                        wkv[:].rearrange("p u c -> p (u c)"),
                        start=True, stop=True)
                kvv = ps_kv[:, :, 0:NU * 17].rearrange(
                    "p it (u c) -> p it u c", u=NU)
                normalize(
                    kvv[:, :, :, 0:D],
                    phiK[:, 0:8, :].rearrange("p d (u it) -> p it u d", u=NU))

                ps_q = pqp.tile([P, NIT, NU * D], F32, tag="pq", name="psq")
                for it in range(NIT):
                    nc.tensor.matmul(
                        ps_q[:, it, :], xT[:, it * P:(it + 1) * P],
                        wq[:].rearrange("p u c -> p (u c)"),
                        start=True, stop=True)
                normalize(
                    ps_q[:].rearrange("p it (u d) -> p it u d", u=NU),
                    phiQ[:, 0:8, :].rearrange("p d (u it) -> p it u d", u=NU))

                nc.scalar.activation(
                    vN[:], kvv[:, :, :, D:].rearrange("p it u c -> p u it c"),
                    AF.Copy)

            # ---- C: monomial features (both sides: 128 kept), k first ----
            import os as _os
            _ab = _os.environ.get("ABLATE", "")
            _nc_feat = 2 if "feat" in _ab else NCOL
            _pool_deg2 = 4 if "pool" in _ab else 0
            _emit_features(nc, phiK, ncol=_nc_feat, pool_deg2=_pool_deg2)
            _emit_features(nc, phiQ, ncol=_nc_feat, pool_deg2=_pool_deg2)

            # ---- pool stack: [trp outp][smp][mtp]; peak 3+2+1+1 = 7 banks.
            fh = ExitStack()
            trp = fh.enter_context(
                tc.tile_pool(name="trp", bufs=4, space="PSUM"))
            outp = fh.enter_context(
                tc.tile_pool(name="outp", bufs=2, space="PSUM"))
            smp_cm = tc.tile_pool(name="smp", bufs=1, space="PSUM")
            smp = smp_cm.__enter__()
            mtp_cm = tc.tile_pool(name="mtp", bufs=1, space="PSUM")
            mtp = mtp_cm.__enter__()
            _skip_mt = "mt" in _ab

            # D: k-side MtT[128f, 9] per unit, accumulated directly in
            # transposed layout: stationary = phiK column tile, moving = vN
            # (9 cols/matmul instead of 128) -- no E-stage transpose needed.
            mtT_ps = mtp.tile([FQ, NU, D + 1], F32, tag="mtT")
            for u in range(NU if not _skip_mt else 0):
                for it in range(NIT):
                    nc.tensor.matmul(
                        mtT_ps[:, u, :], phiK[:, :, u * NIT + it],
                        vN[:, u, it, :],
                        start=(it == 0), stop=(it == NIT - 1))
            if not _skip_mt:
                nc.vector.tensor_copy(mtT_sb[:], mtT_ps[:])
            mtp_cm.__exit__(None, None, None)
            mw_ps = smp.tile([FQ, NU, D + 1], F32, tag="mw")

            def emit_mw():
                # deferred until after the first transpose group so the PE
                # doesn't stall on the DVE mtT copy with transposes queued
                if not _skip_mt:
                    for u in range(NU):
                        nc.tensor.matmul(mw_ps[:, u, :], at1[:, u, :],
                                         mtT_sb[:, u, :], start=True,
                                         stop=True)
                    nc.vector.tensor_copy(Mw[:], mw_ps[:])
                smp_cm.__exit__(None, None, None)

            # ---- F/H fused per unit: transpose group g of phi_q (8 i-tiles)
            # while H consumes group g-1 in NATURAL output layout (stationary
            # = phiT chunk, moving = Mw's 9 cols): out lands on 128
            # partitions so the PSUM->SBUF copies and the final DMA are tiny.
            outN_sb = main.tile([P, NU, NIT, D + 1], BF16, name="outN_sb")

            def emit_h(u, g):
                onat = outp.tile([P, 8, D + 1], F32, tag="onat", name="onat")
                for s in range(8):
                    it = 8 * g + s
                    nc.tensor.matmul(
                        onat[:, s, :], phiT[:, u, it * P:(it + 1) * P],
                        Mw[:, u, :], start=True, stop=True)
                dst = outN_sb[:, u, 8 * g:8 * (g + 1), :]
                # NOTE: Pool/GPSIMD cannot access PSUM on real HW
                if g % 2 == 0:
                    nc.vector.tensor_copy(dst, onat[:])
                else:
                    nc.scalar.activation(dst, onat[:], AF.Copy)

            for u in range(NU if "fg" not in _ab else 0):
                for g in range(4):   # groups of 8 i-tiles
                    tr = trp.tile([P, 8, P], BF16, tag="tr1", name="tr1")
                    for s in range(8):
                        it = 8 * g + s
                        nc.tensor.transpose(
                            tr[:, s, :], phiQ[:, :, u * NIT + it], identB)
                    if g % 2 == 0:
                        nc.vector.tensor_copy(
                            phiT[:, u, g * 8 * P:(g + 1) * 8 * P], tr[:])
                    else:
                        nc.scalar.activation(
                            phiT[:, u, g * 8 * P:(g + 1) * 8 * P],
                            tr[:], AF.Copy)
                    if u == 0 and g == 0:
                        emit_mw()
                    if g >= 1 and "hi" not in _ab:
                        emit_h(u, g - 1)
                if "hi" not in _ab:
                    emit_h(u, 3)
                    (nc.sync if u == 0 else nc.scalar).dma_start(
                        out_d[u], outN_sb[:, u])
            if "fg" in _ab:
                emit_mw()
            fh.close()

            tap_tiles = {"phiK": phiK, "phiQ": phiQ, "vN": vN,
                         "mtT_sb": mtT_sb, "Mw": Mw, "phiT": phiT}
            for tname in taps:
                tl = tap_tiles[tname]
                td = nc.dram_tensor(f"tap_{tname}", list(tl[:].shape),
                                    tl[:].dtype, kind="ExternalOutput").ap()
                nc.sync.dma_start(td, tl[:])

        if reps == 1:
            emit_all()
        else:
            with tc.For_i(0, reps, 1):
                emit_all()

    nc.compile()
    return nc


def _prep_inputs(x, y, Wq, bq, Wkv, bkv, temperature):
    """Host-side sharding/relayout + per-head moment-fit A matrices."""
    x = np.asarray(x, np.float32)
    y = np.asarray(y, np.float32)
    Wq = np.asarray(Wq, np.float32)
    bq = np.asarray(bq, np.float32)
    Wkv = np.asarray(Wkv, np.float32)
    bkv = np.asarray(bkv, np.float32)
    temps = np.asarray(temperature, np.float32).reshape(H)
    projP = _proj_matrix()
    ones = np.ones((1, HW), dtype=np.float32)
    bf = ml_dtypes.bfloat16
    in_maps = []
    for c in range(NCORES):
        b = c // 4
        heads = [2 * (c % 4), 2 * (c % 4) + 1]
        xT = np.concatenate([np.ascontiguousarray(x[b].T), ones], 0)
        yT = np.concatenate([np.ascontiguousarray(y[b].T), ones], 0)
        wq = np.empty((C + 1, NU, D), np.float32)
        wkv = np.zeros((C + 1, NU, 2 * D + 1), np.float32)
        at1 = np.empty((P, NU, FQ), np.float32)
        for u, h in enumerate(heads):
            wq[:C, u, :] = Wq[:, D * h:D * (h + 1)]
            wq[C, u, :] = bq[D * h:D * (h + 1)]
            wkv[:C, u, 0:D] = Wkv[:, D * h:D * (h + 1)]
            wkv[C, u, 0:D] = bkv[D * h:D * (h + 1)]
            wkv[:C, u, D:2 * D] = Wkv[:, C + D * h:C + D * (h + 1)]
            wkv[C, u, D:2 * D] = bkv[C + D * h:C + D * (h + 1)]
            wkv[C, u, 2 * D] = 1.0     # ones column for the denominator
            # two-sided 128-feature bilinear map (see _proj_matrix)
            A2 = (projP * _feat_weights(float(temps[h]))[None, :]) @ projP.T
            at1[:, u, :] = A2.T.astype(np.float32)
        in_maps.append({
            "xT": xT.astype(bf), "yT": yT.astype(bf),
            "wq": wq.astype(bf), "wkv": wkv.astype(bf),
            "at1": at1,
        })
    return in_maps


def _unshard_core0(core_out):
    """core 0 raw out [NU, P, NIT, D+1] -> [HW, 16] float."""
    res = np.empty((HW, NU * D), np.float32)
    for u in range(NU):
        co = np.asarray(core_out[u]).astype(np.float32)
        co = co.transpose(1, 0, 2).reshape(HW, D + 1)
        res[:, D * u:D * (u + 1)] = co[:, :D] / co[:, D:D + 1]
    return res


def run(x, y, Wq, bq, Wkv, bkv, temperature, trace=False):
    if "nc" not in _CACHE:
        _CACHE["nc"] = build_program()
    nc = _CACHE["nc"]
    in_maps = _prep_inputs(x, y, Wq, bq, Wkv, bkv, temperature)
    res = run_bass_kernel_spmd(nc, in_maps, core_ids=list(range(NCORES)),
                               trace=trace)
    out = np.empty((B, HW, C), dtype=np.float32)
    for c in range(NCORES):
        b = c // 4
        heads = [2 * (c % 4), 2 * (c % 4) + 1]
        core_out = res.results[c]["out"]  # [NU, P, NIT, D+1]
        for u, h in enumerate(heads):
            co = np.asarray(core_out[u]).astype(np.float32)
            co = co.transpose(1, 0, 2).reshape(HW, D + 1)
            out[b, :, D * h:D * (h + 1)] = co[:, :D] / co[:, D:D + 1]
    return out, res


def kernel(x, y, Wq, bq, Wkv, bkv, temperature):
    out, _ = run(np.asarray(x), np.asarray(y), np.asarray(Wq), np.asarray(bq),
                 np.asarray(Wkv), np.asarray(bkv), np.asarray(temperature))
    return out


# revision 48
# speedup vs baseline: 1.3086x; 1.1621x over previous
"""Trainium2 Bass kernel for nn_Cross_At_50208167690358 (cosine-sim cross attention).

Math (per reference): q = x@Wq+bq; k,v = y@Wkv+bkv (split); q,k l2-normalized
over head dim (8); attn = softmax((q_hat . k_hat) * temp); out = attn @ v.
B=2, HW=4096, C=64, H=8, hd=8.

Key trick: scores s = q_hat.k_hat are cosine similarities, |s| <= 1.
Replace exp(t*s) by a degree-3 polynomial (Chebyshev interpolant of exp(t*s)
on [-1,1]) which expands exactly over the 165 monomials of degree <= 3 in the
8 head dims, collapsing softmax attention to *linear attention*:

    sum_j p(s_ij) * v_j = phi_q(q_i) . ( A @ Phi_k^T @ V_aug )

TRUNCATED feature maps: both sides keep only 128 of the 165 monomials
(deg<=2 plus deg-3 blocks d=0,1,2 sans (2,7,7); the constant is dropped --
softmax is shift-invariant).  The truncation error is repaired by a full
bilinear map A [128,128], the L2(uniform-sphere)-optimal approximation of
the kernel over the kept features, computed in closed form from monomial
moments: A = Gq^-1 Gqa diag(w(t)) Gqa^T Gq^-1 (per head temperature t).
128 features = exactly one PE-transpose chunk per i-tile and one H matmul
per column block; A folds into the tiny Mw = A @ Mt computation (1 matmul).
Accuracy vs reference on HW (incl bf16 rounding): ~7.0e-3 rel.

Sharding: 16 (b,h) units -> 2 per core (cores share batch b = core // 4).

Per-core pipeline (units fused, bf16 data, fp32 PSUM accumulation):
  A: project k/v/q in natural layout (xT/yT stationary, ones-row bias trick;
     v gets an extra all-ones column for the softmax denominator).
  B: l2-normalize: ACT square/sqrt + DVE reduce/reciprocal/mul (all ACT
     functions stay in the sqrt_and_others table -> one table load).
  C: build monomial features: broadcast-multiplies, all on DVE (the Pool
     engine measured ~4.6x slower per element and slowed the whole kernel).
  D: k-side MtT[128,9] per unit accumulated directly in transposed layout
     (stationary = phiK tile, moving = vN's 9 cols) -- no transpose stage.
  E: Mw[128,9] = A @ MtT (one f32 matmul per unit).
  F/H per unit: PE-transpose Phi_q group g (8 i-tiles) to PSUM (ring of 4),
     copy to SBUF (alternating DVE/ACT), while the H matmuls consume group
     g-1 in NATURAL output layout (stationary = PhiT chunk, moving = Mw's
     9 cols): out[128i,9] lands on 128 partitions, so the PSUM->SBUF copies
     (72 free elems) and the single per-unit out DMA are tiny.
  I: host does num/denom divide + relayout.
"""

import sys

if "/opt/trn_rl_repo" not in sys.path:
    sys.path.insert(0, "/opt/trn_rl_repo")

from contextlib import ExitStack
from math import factorial

import numpy as np
import ml_dtypes

import concourse.bass as bass  # noqa: F401
from concourse import bacc, mybir
import concourse.tile as tile
from concourse.bass_utils import run_bass_kernel_spmd
from concourse.masks import make_identity

P = 128
HW = 4096
C = 64
H = 8
D = 8          # head dim
B = 2
NCORES = 8
NU = 2         # (b, h) units per core
NIT = HW // P  # 32 i-tiles
NCOL = NU * NIT  # 64 fused (unit, i-tile) columns

DEG = 3
F = 165        # k-side features: 1 + 8 + 36 + 120 monomials up to degree 3
FQ = 128       # q-side features (see module docstring)
F2 = F - 128   # second k-chunk width for the A matmul (37)
IC = 1024      # H-stage column block (= one 8-i-tile transpose group)
NCH = HW // IC

F32 = mybir.dt.float32
BF16 = mybir.dt.bfloat16
AF = mybir.ActivationFunctionType

# k-side feature-block offsets (degree-2 / degree-3 prefix tables)
W2 = [8 - d for d in range(8)]                      # widths of deg-2 blocks
OFF2 = [9 + sum(W2[:d]) for d in range(8)]          # deg-2 block starts
W3 = [sum(W2[d:]) for d in range(8)]                # widths of deg-3 blocks
OFF3 = [45 + sum(W3[:d]) for d in range(8)]         # deg-3 block starts
assert OFF3[-1] + W3[-1] == F

# q-side kept monomials: deg1 (8) + deg2 (36) + deg3 blocks d=0,1,2 with the
# final (2,7,7) monomial trimmed (36+28+20).  Offsets within the 128 rows:
OFF2Q = [o - 1 for o in OFF2]                       # deg2 starts (base 8)
W3Q = [36, 28, 20]
OFF3Q = [44, 80, 108]
assert OFF3Q[-1] + W3Q[-1] == FQ
# indices of kept q features inside the 165-dim monomial order
KEPT_Q = list(range(1, 130))
KEPT_Q.remove(129)   # (2,7,7)
assert len(KEPT_Q) == FQ

_CACHE = {}


def _feat_weights(t):
    """Poly-kernel weights w_f so that sum_f w_f phi_f(q) phi_f(k) ~ exp(t*q.k)
    for unit q, k. Chebyshev interpolant of exp(t*s) on [-1,1], degree 3."""
    cheb = np.polynomial.chebyshev.chebinterpolate(
        lambda s: np.exp(t * s), DEG)
    c = np.polynomial.chebyshev.cheb2poly(cheb)

    def multinom(idx):
        counts = {}
        for d in idx:
            counts[d] = counts.get(d, 0) + 1
        r = factorial(len(idx))
        for v in counts.values():
            r //= factorial(v)
        return r

    w = np.empty(F, np.float64)
    w[0] = c[0]
    for d in range(8):
        w[1 + d] = c[1]
    i = 9
    for d1 in range(8):
        for d2 in range(d1, 8):
            w[i] = c[2] * multinom((d1, d2))
            i += 1
    for d1 in range(8):
        for d2 in range(d1, 8):
            for d3 in range(d2, 8):
                w[i] = c[3] * multinom((d1, d2, d3))
                i += 1
    assert i == F
    return w


def _monomial_exponents():
    exps = [(0,) * 8]
    for d in range(8):
        e = [0] * 8
        e[d] = 1
        exps.append(tuple(e))
    for d1 in range(8):
        for d2 in range(d1, 8):
            e = [0] * 8
            e[d1] += 1
            e[d2] += 1
            exps.append(tuple(e))
    for d1 in range(8):
        for d2 in range(d1, 8):
            for d3 in range(d2, 8):
                e = [0] * 8
                e[d1] += 1
                e[d2] += 1
                e[d3] += 1
                exps.append(tuple(e))
    return np.array(exps)


def _proj_matrix():
    """P = Gq^-1 Gqa from uniform-S^7 monomial moments; A(t) = P diag(w(t)).
    Minimizes E_{q,k uniform}[(phiq^T A phik - p_t(q.k))^2]."""
    if "projP" in _CACHE:
        return _CACHE["projP"]
    exps = _monomial_exponents()

    def dfact(n):
        r = 1
        while n > 1:
            r *= n
            n -= 2
        return r

    def moment(a):
        if any(x % 2 for x in a):
            return 0.0
        num = 1.0
        for x in a:
            num *= dfact(x - 1)
        den = 1.0
        for j in range(sum(a) // 2):
            den *= (8 + 2 * j)
        return num / den

    G = np.zeros((F, F))
    for i in range(F):
        for j in range(i, F):
            G[i, j] = G[j, i] = moment(exps[i] + exps[j])
    Gq = G[np.ix_(KEPT_Q, KEPT_Q)]
    Gqa = G[KEPT_Q, :]
    _CACHE["projP"] = np.linalg.solve(Gq, Gqa)
    return _CACHE["projP"]


def _emit_features(nc, phi, ncol=NCOL, pool_deg2=4):
    """Monomials for the 128 kept features (both sides): no const; linear at
    rows 0:8; deg2 at 8:44; deg3 blocks d=0,1,2 at 44:128 ((2,7,7) trimmed).
    deg-2 blocks d < pool_deg2 go to the Pool engine (idle but slower); the
    rest + all deg-3 go to DVE.  d descending so deg-3 block d (reading the
    deg-2 suffix from block d) can start early."""
    def hat(d):
        return phi[:, d:d + 1, 0:ncol]

    for d in reversed(range(8)):
        w = 8 - d
        eng = nc.gpsimd if d < pool_deg2 else nc.vector
        eng.tensor_mul(
            phi[:, OFF2Q[d]:OFF2Q[d] + w, 0:ncol],
            hat(d).to_broadcast((P, w, ncol)),
            phi[:, d:8, 0:ncol])
    for d in reversed(range(3)):
        w = W3Q[d]
        nc.vector.tensor_mul(
            phi[:, OFF3Q[d]:OFF3Q[d] + w, 0:ncol],
            hat(d).to_broadcast((P, w, ncol)),
            phi[:, OFF2Q[d]:OFF2Q[d] + w, 0:ncol])


def build_program(reps=1, taps=()):
    nc = bacc.Bacc("TRN2", target_bir_lowering=False, debug=False,
                   num_devices=NCORES)
    xT_d = nc.dram_tensor("xT", [C + 1, HW], BF16, kind="ExternalInput").ap()
    yT_d = nc.dram_tensor("yT", [C + 1, HW], BF16, kind="ExternalInput").ap()
    wq_d = nc.dram_tensor("wq", [C + 1, NU, D], BF16, kind="ExternalInput").ap()
    wkv_d = nc.dram_tensor("wkv", [C + 1, NU, 2 * D + 1], BF16,
                           kind="ExternalInput").ap()
    at1_d = nc.dram_tensor("at1", [P, NU, FQ], F32, kind="ExternalInput").ap()
    out_d = nc.dram_tensor("out", [NU, P, NIT, D + 1], BF16,
                           kind="ExternalOutput").ap()

    with tile.TileContext(nc) as tc, ExitStack() as ctx:
        pools = {
            "const": ctx.enter_context(tc.tile_pool(name="const", bufs=1)),
            "main": ctx.enter_context(tc.tile_pool(name="main", bufs=1)),
            "work": ctx.enter_context(tc.tile_pool(name="work", bufs=2)),
        }

        def emit_all():
            const, main, work = pools["const"], pools["main"], pools["work"]
            xT = const.tile([C + 1, HW], BF16, name="xT")
            yT = const.tile([C + 1, HW], BF16, name="yT")
            wq = const.tile([C + 1, NU, D], BF16, name="wq")
            wkv = const.tile([C + 1, NU, 2 * D + 1], BF16, name="wkv")
            at1 = const.tile([P, NU, FQ], F32, name="at1")
            identB = const.tile([P, P], BF16, name="identB")
            # weights first (tiny), then bulk inputs split across both HWDGE
            # queues (SP + ACT) in halves so projections can start early.
            nc.sync.dma_start(wkv[:], wkv_d)
            nc.sync.dma_start(wq[:], wq_d)
            nc.scalar.dma_start(at1[:], at1_d)
            HQ = HW // 4
            for qq in range(4):
                eng = nc.sync if qq % 2 == 0 else nc.scalar
                eng.dma_start(yT[:, qq * HQ:(qq + 1) * HQ],
                              yT_d[:, qq * HQ:(qq + 1) * HQ])
            for qq in range(4):
                eng = nc.scalar if qq % 2 == 0 else nc.sync
                eng.dma_start(xT[:, qq * HQ:(qq + 1) * HQ],
                              xT_d[:, qq * HQ:(qq + 1) * HQ])
            make_identity(nc, identB[:])

            # Pin the ACT function table: a no-dep Sqrt schedules first, so
            # the greedy table chooser picks sqrt_and_others (which also
            # holds Copy + Square) once instead of thrashing 1.28us reloads.
            actpin = work.tile([1, 1], F32, tag="actpin")
            nc.scalar.activation(actpin[:], identB[0:1, 0:1], AF.Sqrt)

            phiK = main.tile([P, FQ, NCOL], BF16, name="phiK")
            phiQ = main.tile([P, FQ, NCOL], BF16, name="phiQ")
            vN = main.tile([P, NU, NIT, D + 1], BF16, name="vN")
            phiT = main.tile([FQ, NU, HW], BF16, name="phiT")
            mtT_sb = main.tile([P, NU, D + 1], F32, name="mtT_sb")
            Mw = main.tile([FQ, NU, D + 1], BF16, name="Mw")

            def normalize(psv, phi_slots):
                # psv: [P, NIT, NU, 8] projection view (PSUM fp32).
                # Square/Sqrt on ACT + reduce/recip/mul on DVE: every ACT
                # function stays in the sqrt_and_others table.
                sq = work.tile([P, NIT, NU, D], F32, tag="sq")
                nc.scalar.activation(sq[:], psv, AF.Square)
                ssum = work.tile([P, NIT, NU], F32, tag="ssum")
                nc.vector.tensor_reduce(ssum[:], sq[:], mybir.AxisListType.X,
                                        mybir.AluOpType.add)
                rec = work.tile([P, NIT, NU], F32, tag="rec")
                nc.vector.reciprocal(rec[:], ssum[:])
                inv = work.tile([P, NIT, NU], F32, tag="inv")
                nc.scalar.activation(inv[:], rec[:], AF.Sqrt)
                nc.vector.tensor_mul(
                    phi_slots, psv,
                    inv[:, :, :, None].to_broadcast((P, NIT, NU, D)))

            # ---- A: projections, k first (feeds the longest chain:
            # featK -> Mt -> Mw).  k+v fused: one matmul per i-tile with
            # the yT tile stationary (34 cols: per unit 8 k + 9 v-aug).
            with tc.tile_pool(name="pkv", bufs=1, space="PSUM") as pkvp, \
                    tc.tile_pool(name="pq", bufs=1, space="PSUM") as pqp:
                ps_kv = pkvp.tile([P, NIT, 64], F32, tag="pkv", name="pskv")
                for it in range(NIT):
                    nc.tensor.matmul(
                        ps_kv[:, it, 0:NU * 17],
                        yT[:, it * P:(it + 1) * P],
                        wkv[:].rearrange("p u c -> p (u c)"),
                        start=True, stop=True)
                kvv = ps_kv[:, :, 0:NU * 17].rearrange(
                    "p it (u c) -> p it u c", u=NU)
                normalize(
                    kvv[:, :, :, 0:D],
                    phiK[:, 0:8, :].rearrange("p d (u it) -> p it u d", u=NU))

                ps_q = pqp.tile([P, NIT, NU * D], F32, tag="pq", name="psq")
                for it in range(NIT):
                    nc.tensor.matmul(
                        ps_q[:, it, :], xT[:, it * P:(it + 1) * P],
                        wq[:].rearrange("p u c -> p (u c)"),
                        start=True, stop=True)
                normalize(
                    ps_q[:].rearrange("p it (u d) -> p it u d", u=NU),
                    phiQ[:, 0:8, :].rearrange("p d (u it) -> p it u d", u=NU))

                nc.scalar.activation(
                    vN[:], kvv[:, :, :, D:].rearrange("p it u c -> p u it c"),
                    AF.Copy)

            # ---- C: monomial features (both sides: 128 kept), k first ----
            import os as _os
            _ab = _os.environ.get("ABLATE", "")
            _nc_feat = 2 if "feat" in _ab else NCOL
            _pool_deg2 = 4 if "pool" in _ab else 0
            _emit_features(nc, phiK, ncol=_nc_feat, pool_deg2=_pool_deg2)
            _emit_features(nc, phiQ, ncol=_nc_feat, pool_deg2=_pool_deg2)

            # ---- pool stack: [trp outp][smp][mtp]; peak 3+2+1+1 = 7 banks.
            fh = ExitStack()
            trp = fh.enter_context(
                tc.tile_pool(name="trp", bufs=4, space="PSUM"))
            outp = fh.enter_context(
                tc.tile_pool(name="outp", bufs=2, space="PSUM"))
            smp_cm = tc.tile_pool(name="smp", bufs=1, space="PSUM")
            smp = smp_cm.__enter__()
            mtp_cm = tc.tile_pool(name="mtp", bufs=1, space="PSUM")
            mtp = mtp_cm.__enter__()
            _skip_mt = "mt" in _ab

            # D: k-side MtT[128f, 9] per unit, accumulated directly in
            # transposed layout: stationary = phiK column tile, moving = vN
            # (9 cols/matmul instead of 128) -- no E-stage transpose needed.
            mtT_ps = mtp.tile([FQ, NU, D + 1], F32, tag="mtT")
            for u in range(NU if not _skip_mt else 0):
                for it in range(NIT):
                    nc.tensor.matmul(
                        mtT_ps[:, u, :], phiK[:, :, u * NIT + it],
                        vN[:, u, it, :],
                        start=(it == 0), stop=(it == NIT - 1))
            if not _skip_mt:
                nc.vector.tensor_copy(mtT_sb[:], mtT_ps[:])
            mtp_cm.__exit__(None, None, None)
            mw_ps = smp.tile([FQ, NU, D + 1], F32, tag="mw")

            def emit_mw():
                # deferred until after the first transpose group so the PE
                # doesn't stall on the DVE mtT copy with transposes queued
                if not _skip_mt:
                    for u in range(NU):
                        nc.tensor.matmul(mw_ps[:, u, :], at1[:, u, :],
                                         mtT_sb[:, u, :], start=True,
                                         stop=True)
                    nc.vector.tensor_copy(Mw[:], mw_ps[:])
                smp_cm.__exit__(None, None, None)

            # ---- F/H fused per unit: transpose group g of phi_q (8 i-tiles)
            # while H consumes group g-1 in NATURAL output layout (stationary
            # = phiT chunk, moving = Mw's 9 cols): out lands on 128
            # partitions so the PSUM->SBUF copies and the final DMA are tiny.
            outN_sb = main.tile([P, NU, NIT, D + 1], BF16, name="outN_sb")

            def emit_h(u, g):
                onat = outp.tile([P, 8, D + 1], F32, tag="onat", name="onat")
                for s in range(8):
                    it = 8 * g + s
                    nc.tensor.matmul(
                        onat[:, s, :], phiT[:, u, it * P:(it + 1) * P],
                        Mw[:, u, :], start=True, stop=True)
                dst = outN_sb[:, u, 8 * g:8 * (g + 1), :]
                # NOTE: Pool/GPSIMD cannot access PSUM on real HW
                if g % 2 == 0:
                    nc.vector.tensor_copy(dst, onat[:])
                else:
                    nc.scalar.activation(dst, onat[:], AF.Copy)

            for u in range(NU if "fg" not in _ab else 0):
                for g in range(4):   # groups of 8 i-tiles
                    tr = trp.tile([P, 8, P], BF16, tag="tr1", name="tr1")
                    for s in range(8):
                        it = 8 * g + s
                        nc.tensor.transpose(
                            tr[:, s, :], phiQ[:, :, u * NIT + it], identB)
                    if g % 2 == 0:
                        nc.vector.tensor_copy(
                            phiT[:, u, g * 8 * P:(g + 1) * 8 * P], tr[:])
                    else:
                        nc.scalar.activation(
                            phiT[:, u, g * 8 * P:(g + 1) * 8 * P],
                            tr[:], AF.Copy)
                    if u == 0 and g == 0:
                        emit_mw()
                    if g >= 1 and "hi" not in _ab:
                        emit_h(u, g - 1)
                if "hi" not in _ab:
                    emit_h(u, 3)
                    (nc.sync if u == 0 else nc.scalar).dma_start(
                        out_d[u], outN_sb[:, u])
            if "fg" in _ab:
                emit_mw()
            fh.close()

            tap_tiles = {"phiK": phiK, "phiQ": phiQ, "vN": vN,
                         "mtT_sb": mtT_sb, "Mw": Mw, "phiT": phiT}
            for tname in taps:
                tl = tap_tiles[tname]
                td = nc.dram_tensor(f"tap_{tname}", list(tl[:].shape),
                                    tl[:].dtype, kind="ExternalOutput").ap()
                nc.sync.dma_start(td, tl[:])

        if reps == 1:
            emit_all()
        else:
            with tc.For_i(0, reps, 1):
                emit_all()

    nc.compile()
    return nc


def _prep_inputs(x, y, Wq, bq, Wkv, bkv, temperature):
    """Host-side sharding/relayout + per-head moment-fit A matrices."""
    x = np.asarray(x, np.float32)
    y = np.asarray(y, np.float32)
    Wq = np.asarray(Wq, np.float32)
    bq = np.asarray(bq, np.float32)
    Wkv = np.asarray(Wkv, np.float32)
    bkv = np.asarray(bkv, np.float32)
    temps = np.asarray(temperature, np.float32).reshape(H)
    projP = _proj_matrix()
    ones = np.ones((1, HW), dtype=np.float32)
    bf = ml_dtypes.bfloat16
    in_maps = []
    for c in range(NCORES):
        b = c // 4
        heads = [2 * (c % 4), 2 * (c % 4) + 1]
        xT = np.concatenate([np.ascontiguousarray(x[b].T), ones], 0)
        yT = np.concatenate([np.ascontiguousarray(y[b].T), ones], 0)
        wq = np.empty((C + 1, NU, D), np.float32)
        wkv = np.zeros((C + 1, NU, 2 * D + 1), np.float32)
        at1 = np.empty((P, NU, FQ), np.float32)
        for u, h in enumerate(heads):
            wq[:C, u, :] = Wq[:, D * h:D * (h + 1)]
            wq[C, u, :] = bq[D * h:D * (h + 1)]
            wkv[:C, u, 0:D] = Wkv[:, D * h:D * (h + 1)]
            wkv[C, u, 0:D] = bkv[D * h:D * (h + 1)]
            wkv[:C, u, D:2 * D] = Wkv[:, C + D * h:C + D * (h + 1)]
            wkv[C, u, D:2 * D] = bkv[C + D * h:C + D * (h + 1)]
            wkv[C, u, 2 * D] = 1.0     # ones column for the denominator
            # two-sided 128-feature bilinear map (see _proj_matrix)
            A2 = (projP * _feat_weights(float(temps[h]))[None, :]) @ projP.T
            at1[:, u, :] = A2.T.astype(np.float32)
        in_maps.append({
            "xT": xT.astype(bf), "yT": yT.astype(bf),
            "wq": wq.astype(bf), "wkv": wkv.astype(bf),
            "at1": at1,
        })
    return in_maps


def _unshard_core0(core_out):
    """core 0 raw out [NU, P, NIT, D+1] -> [HW, 16] float."""
    res = np.empty((HW, NU * D), np.float32)
    for u in range(NU):
        co = np.asarray(core_out[u]).astype(np.float32)
        co = co.transpose(1, 0, 2).reshape(HW, D + 1)
        res[:, D * u:D * (u + 1)] = co[:, :D] / co[:, D:D + 1]
    return res


def run(x, y, Wq, bq, Wkv, bkv, temperature, trace=False):
    if "nc" not in _CACHE:
        _CACHE["nc"] = build_program()
    nc = _CACHE["nc"]
    in_maps = _prep_inputs(x, y, Wq, bq, Wkv, bkv, temperature)
    res = run_bass_kernel_spmd(nc, in_maps, core_ids=list(range(NCORES)),
                               trace=trace)
    out = np.empty((B, HW, C), dtype=np.float32)
    for c in range(NCORES):
        b = c // 4
        heads = [2 * (c % 4), 2 * (c % 4) + 1]
        core_out = res.results[c]["out"]  # [NU, P, NIT, D+1]
        for u, h in enumerate(heads):
            co = np.asarray(core_out[u]).astype(np.float32)
            co = co.transpose(1, 0, 2).reshape(HW, D + 1)
            out[b, :, D * h:D * (h + 1)] = co[:, :D] / co[:, D:D + 1]
    return out, res


def kernel(x, y, Wq, bq, Wkv, bkv, temperature):
    out, _ = run(np.asarray(x), np.asarray(y), np.asarray(Wq), np.asarray(bq),
                 np.asarray(Wkv), np.asarray(bkv), np.asarray(temperature))
    return out
